# revision 23
# baseline (speedup 1.0000x reference)
"""Trainium2 Bass kernel for nn_EnergyOut (per-node MLP + segment_sum).

Computation (reference):
    h = silu(node_scalar @ W1.T + b1)        # [N, 64]
    atom = h @ W2.T + b2                     # [N]
    atomic_energies = atomic_energies_in + atom
    total_energy = segment_sum(atomic_energies, batch, 16384)

Strategy (8 NeuronCores, data-parallel over nodes):
  - Host marshalling: each core gets 62500 nodes (padded to 63488 = 31*2048).
    x is shipped pre-transposed and bf16-cast as xT [128(d), 63488(n)] so the
    contraction dim d lands on SBUF partitions with contiguous DMA runs.
    Per-chunk relative batch offsets (REL, exploiting sortedness) ship re-laid
    as [128, 496] (node n = chunk*128 + p -> column chunk, partition p).
  - Device, per 2048-node macro-tile (16 chunks of 128):
      PE:  rank-1 matmul (ones x b1row) pre-biases PSUM with b1, then 16
           chunked matmuls lhsT=xT_chunk [d=128, n=128] x W1T [128,64]
           accumulate h+b1 into PSUM [128(node), (c,h)=1024].
      ACT: silu PSUM -> SBUF bf16.
      DVE: *W2rep (tensor_tensor bf16), strided reduce over h -> e_mlp
           [128, 16] written into the resident out_e tile (f32),
           one-hot A[p,(c,j)] = is_equal(iota, REL) [128, (16,32)=512].
      PE:  4 segment matmuls lhsT=A[:, q*128:+128], rhs=e slice [128,4] ->
           PSUM [128, 496] partials (diag blocks = per-chunk 32-bin sums;
           sorted batch => a 128-node chunk spans < 32 graphs w.h.p.).
  - Host: adds e_in+b2 (exact) and its input-only bincount to totals;
    scatter-adds the 32-wide per-chunk partials into the 16384 graph bins.
"""

import sys

import numpy as np

try:
    import concourse.bass as bass  # noqa: F401
except ImportError:
    sys.path.insert(0, "/opt/trn_rl_repo")

import ml_dtypes
import concourse.bass as bass
import concourse.bacc as bacc
import concourse.tile as tile
from concourse import mybir
from concourse.bass_utils import run_bass_kernel_spmd

BF16 = mybir.dt.bfloat16
F32 = mybir.dt.float32

N_NODES = 500_000
NODE_DIM = 128
HIDDEN = 64
N_GRAPHS = 16_384
N_CORES = 8
NPC = N_NODES // N_CORES          # 62500 nodes per core
CHUNK = 128                       # nodes per chunk (PE contraction width)
MACRO = 2048                      # nodes per macro-tile
CPM = MACRO // CHUNK              # 16 chunks per macro
N_MACRO = (NPC + MACRO - 1) // MACRO   # 31
NPAD = N_MACRO * MACRO            # 63488
N_CHUNK = NPAD // CHUNK           # 496
SPAN = 32                         # max graphs spanned by one sorted 128-chunk


def build_program(level=6):
    """Build the single-core Bass/Tile program (same program on all 8 cores).

    level: cumulative stage count for timeline-sim bisection.
      0: x DMA only; 1: +mm1; 2: +silu; 3: +mm2; 5: +aall; 6: +segmm
    """
    nc = bacc.Bacc("TRN2", target_bir_lowering=False, debug=False,
                   num_devices=N_CORES)

    HW = CPM * HIDDEN  # 1024 free elems of h per macro

    # DRAM I/O
    xT = nc.dram_tensor("xT", [NODE_DIM, NPAD], BF16, kind="ExternalInput").ap()
    w1t = nc.dram_tensor("w1t", [NODE_DIM, HIDDEN], BF16, kind="ExternalInput").ap()
    w2rep = nc.dram_tensor("w2rep", [CHUNK, HW], BF16, kind="ExternalInput").ap()
    b1row = nc.dram_tensor("b1row", [1, HW], BF16, kind="ExternalInput").ap()
    ones1 = nc.dram_tensor("ones1", [1, CHUNK], BF16, kind="ExternalInput").ap()
    FP8 = mybir.dt.float8e4
    aoh = nc.dram_tensor("aoh", [CHUNK, N_CHUNK * SPAN], FP8, kind="ExternalInput").ap()
    out_e = nc.dram_tensor("out_e", [CHUNK, N_CHUNK], F32, kind="ExternalOutput").ap()
    out_p = nc.dram_tensor("out_p", [CHUNK, N_CHUNK], F32, kind="ExternalOutput").ap()

    with tile.TileContext(nc) as tc:
        with (
            tc.tile_pool(name="res", bufs=1) as res,
            tc.tile_pool(name="xin", bufs=4) as xin,
            tc.tile_pool(name="silu", bufs=3) as spool,
            tc.tile_pool(name="prod", bufs=3) as ppool,
            tc.tile_pool(name="aall", bufs=3) as apool,
            tc.tile_pool(name="hps", bufs=3, space="PSUM") as hps,
            tc.tile_pool(name="pps", bufs=1, space="PSUM") as pps,
        ):
            # Resident tiles
            t_w1t = res.tile([NODE_DIM, HIDDEN], BF16)
            t_w2rep = res.tile([CHUNK, HW], BF16)
            t_b1row = res.tile([1, HW], BF16)
            t_ones1 = res.tile([1, CHUNK], BF16)
            t_aoh = res.tile([CHUNK, N_CHUNK * SPAN], FP8)
            t_oute = res.tile([CHUNK, N_CHUNK], F32)
            t_psb = res.tile([CHUNK, N_CHUNK], F32)
            psum_p = pps.tile([CHUNK, N_CHUNK], F32)

            nc.sync.dma_start(t_w1t[:], w1t)
            nc.sync.dma_start(t_w2rep[:], w2rep)
            nc.sync.dma_start(t_b1row[:], b1row)
            nc.sync.dma_start(t_ones1[:], ones1)
            for q4 in range(4):
                qs = N_CHUNK * SPAN // 4
                nc.sync.dma_start(t_aoh[:, q4 * qs:(q4 + 1) * qs],
                                  aoh[:, q4 * qs:(q4 + 1) * qs])

            for m in range(N_MACRO):
                t_x = xin.tile([NODE_DIM, MACRO], BF16)
                nc.sync.dma_start(t_x[:], xT[:, m * MACRO:(m + 1) * MACRO])

                if level < 1:
                    continue
                # h = x @ W1T + b1 into PSUM, nodes on partitions.
                ps_h = hps.tile([CHUNK, HW], F32)
                # rank-1 bias fill: out[p, (c,h)] = ones[p] * b1row[(c,h)]
                # (split at the PSUM bank boundary, N<=512 f32 per matmul)
                for u in range(HW // 512):
                    nc.tensor.matmul(ps_h[:, u * 512:(u + 1) * 512],
                                     t_ones1[:], t_b1row[:, u * 512:(u + 1) * 512],
                                     start=True, stop=False,
                                     skip_group_check=True)
                for c in range(CPM):
                    nc.tensor.matmul(
                        ps_h[:, c * HIDDEN:(c + 1) * HIDDEN],
                        t_x[:, c * CHUNK:(c + 1) * CHUNK],
                        t_w1t[:],
                        start=False, stop=(c == CPM - 1),
                        skip_group_check=True)

                if level < 2:
                    continue
                t_silu = spool.tile([CHUNK, HW], BF16)
                nc.scalar.activation(t_silu[:], ps_h[:],
                                     mybir.ActivationFunctionType.Silu)

                if level < 3:
                    continue
                # e_mlp per node into the resident output tile (e_in + b2 is
                # added on the host; the device segments the MLP part only).
                t_prod = ppool.tile([CHUNK, HW], BF16)
                oute_sl = t_oute[:, m * CPM:(m + 1) * CPM]
                nc.vector.tensor_mul(t_prod[:], t_silu[:], t_w2rep[:])
                nc.vector.tensor_reduce(
                    oute_sl,
                    t_prod[:].rearrange("p (c h) -> p c h", h=HIDDEN),
                    axis=mybir.AxisListType.X, op=mybir.AluOpType.add)

                if level < 5:
                    continue
                # bf16 copy of e for the all-bf16 segment matmul
                t_e16 = apool.tile([CHUNK, CPM], BF16)
                nc.vector.tensor_copy(t_e16[:], oute_sl)

                if level < 6:
                    continue
                # segment partials, 4 chunks per matmul:
                # out[(c,j), c'] = sum_p A[p, (c,j)] e[p, c']
                for q in range(CPM // 4):
                    k0 = m * CPM + 4 * q
                    nc.tensor.matmul(
                        psum_p[:, k0:k0 + 4],
                        t_aoh[:, k0 * SPAN:(k0 + 4) * SPAN],
                        t_e16[:, 4 * q:4 * (q + 1)],
                        start=True, stop=True, skip_group_check=True)

            nc.vector.tensor_copy(t_psb[:], psum_p[:])
            nc.sync.dma_start(out_e, t_oute[:])
            nc.sync.dma_start(out_p, t_psb[:])

    nc.compile()
    return nc


_NC_CACHE = None


def _get_program():
    global _NC_CACHE
    if _NC_CACHE is None:
        _NC_CACHE = build_program()
    return _NC_CACHE


def _prep_core_inputs(xs, bs, W1, b1, W2):
    """Host-side marshalling for one core's shard (xs [NPC,128] f32,
    bs [NPC] int). Returns (in_map, g_base, span_ok)."""
    bf16 = ml_dtypes.bfloat16

    xT = np.zeros((NODE_DIM, NPAD), dtype=bf16)
    xT[:, :NPC] = np.ascontiguousarray(xs.astype(bf16).T)

    rel = np.full(NPAD, -1.0, dtype=np.float32)
    g_base = np.full(N_CHUNK, -1, dtype=np.int64)
    bs64 = bs.astype(np.int64)
    n_valid_chunks = (NPC + CHUNK - 1) // CHUNK
    span_ok = True
    for k in range(n_valid_chunks):
        s = k * CHUNK
        e = min(s + CHUNK, NPC)
        base = bs64[s]
        g_base[k] = base
        r = bs64[s:e] - base
        if r.max() >= SPAN:
            span_ok = False
        rel[s:e] = r.astype(np.float32)
    rel_t = rel.reshape(N_CHUNK, CHUNK).T                      # [128, 496]
    aoh = (rel_t[:, :, None] == np.arange(SPAN, dtype=np.float32)
           ).astype(ml_dtypes.float8_e4m3fn).reshape(CHUNK, N_CHUNK * SPAN)

    w1t = np.ascontiguousarray(W1.T).astype(bf16)               # [128, 64]
    w2row = np.tile(np.asarray(W2).reshape(HIDDEN), CPM)        # [1024]
    w2rep = np.broadcast_to(w2row, (CHUNK, CPM * HIDDEN)).astype(bf16).copy()
    b1row = np.tile(np.asarray(b1), CPM).reshape(1, -1).astype(bf16)
    ones1 = np.ones((1, CHUNK), dtype=bf16)
    in_map = dict(xT=xT, aoh=aoh, w1t=w1t, w2rep=w2rep,
                  b1row=b1row, ones1=ones1)
    return in_map, g_base, span_ok


def kernel(node_scalar, atomic_energies_in, batch, W1, b1, W2, b2):
    node_scalar = np.asarray(node_scalar, dtype=np.float32)
    atomic_energies_in = np.asarray(atomic_energies_in, dtype=np.float32)
    batch = np.asarray(batch)
    W1 = np.asarray(W1, dtype=np.float32)
    b1 = np.asarray(b1, dtype=np.float32)
    W2 = np.asarray(W2, dtype=np.float32)
    b2 = np.asarray(b2, dtype=np.float32)

    nc = _get_program()

    in_maps = []
    g_bases = []
    span_ok_all = True
    for i in range(N_CORES):
        sl = slice(i * NPC, (i + 1) * NPC)
        in_map, g_base, span_ok = _prep_core_inputs(
            node_scalar[sl], batch[sl], W1, b1, W2)
        in_maps.append(in_map)
        g_bases.append(g_base)
        span_ok_all = span_ok_all and span_ok

    res = run_bass_kernel_spmd(nc, in_maps, list(range(N_CORES)))
    results = res.results

    # device computed the MLP term only; e_in + b2 is added here (exact),
    # and its (input-only) segment contribution via bincount.
    ein_b2 = atomic_energies_in + np.float32(b2[0])
    batch64 = batch.astype(np.int64)

    atomic = np.empty(N_NODES, dtype=np.float32)
    total = np.zeros(N_GRAPHS, dtype=np.float64)

    jj = np.arange(SPAN)
    kk = np.arange(N_CHUNK)
    for i in range(N_CORES):
        oe = np.asarray(results[i]["out_e"])        # [128, 496]
        atomic[i * NPC:(i + 1) * NPC] = (
            np.ascontiguousarray(oe.T).reshape(-1)[:NPC])

        if span_ok_all:
            op = np.asarray(results[i]["out_p"])    # [128, 496]
            # partial for chunk k, offset j sits at op[(k % 4)*32 + j, k]
            vals = op[(kk % 4)[:, None] * SPAN + jj[None, :], kk[:, None]]
            gb = g_bases[i]
            valid = gb >= 0
            idx = np.minimum(gb[valid, None] + jj[None, :], N_GRAPHS - 1)
            np.add.at(total, idx.ravel(),
                      vals[valid].astype(np.float64).ravel())

    atomic += ein_b2

    if span_ok_all:
        total += np.bincount(batch64, weights=ein_b2, minlength=N_GRAPHS)
    else:
        # Pathological batch distribution (a 128-node chunk spanning >= 32
        # graphs): host segment-sum of the device-computed atomic energies.
        total = np.zeros(N_GRAPHS, dtype=np.float64)
        np.add.at(total, batch64, atomic.astype(np.float64))

    return atomic, total.astype(np.float32)


if __name__ == "__main__":
    # smoke test vs a local numpy reference
    rng = np.random.default_rng(0)
    x = rng.standard_normal((N_NODES, NODE_DIM), dtype=np.float32)
    ein = rng.standard_normal(N_NODES, dtype=np.float32)
    b = np.sort(rng.integers(0, N_GRAPHS, N_NODES))
    W1_ = rng.uniform(-0.09, 0.09, (HIDDEN, NODE_DIM)).astype(np.float32)
    b1_ = rng.uniform(-0.09, 0.09, HIDDEN).astype(np.float32)
    W2_ = rng.uniform(-0.125, 0.125, (1, HIDDEN)).astype(np.float32)
    b2_ = np.zeros(1, dtype=np.float32)
    a, t = kernel(x, ein, b, W1_, b1_, W2_, b2_)
    h = x @ W1_.T + b1_
    h = h * (1.0 / (1.0 + np.exp(-h)))
    atom = h @ W2_.reshape(-1) + b2_[0] + ein
    tot = np.zeros(N_GRAPHS)
    np.add.at(tot, b, atom)
    print("atomic rel err:", np.abs(a - atom).max() / np.abs(atom).max())
    print("total rel err:", np.abs(t - tot).max() / np.abs(tot).max())


# revision 24
# speedup vs baseline: 8.8887x; 8.8887x over previous
"""Trainium2 Bass kernel for nn_EnergyOut (per-node MLP + segment_sum).

Computation (reference):
    h = silu(node_scalar @ W1.T + b1)        # [N, 64]
    atom = h @ W2.T + b2                     # [N]
    atomic_energies = atomic_energies_in + atom
    total_energy = segment_sum(atomic_energies, batch, 16384)

Strategy (8 NeuronCores, data-parallel over nodes):
  - Host marshalling: each core gets 62500 nodes (padded to 63488 = 31*2048).
    x is shipped pre-transposed and bf16-cast as xT [128(d), 63488(n)] so the
    contraction dim d lands on SBUF partitions with contiguous DMA runs.
    Per-chunk relative batch offsets (REL, exploiting sortedness) ship re-laid
    as [128, 496] (node n = chunk*128 + p -> column chunk, partition p).
  - Device, per 2048-node macro-tile (16 chunks of 128):
      PE:  rank-1 matmul (ones x b1row) pre-biases PSUM with b1, then 16
           chunked matmuls lhsT=xT_chunk [d=128, n=128] x W1T [128,64]
           accumulate h+b1 into PSUM [128(node), (c,h)=1024].
      ACT: silu PSUM -> SBUF bf16.
      DVE: *W2rep (tensor_tensor bf16), strided reduce over h -> e_mlp
           [128, 16] written into the resident out_e tile (f32),
           one-hot A[p,(c,j)] = is_equal(iota, REL) [128, (16,32)=512].
      PE:  4 segment matmuls lhsT=A[:, q*128:+128], rhs=e slice [128,4] ->
           PSUM [128, 496] partials (diag blocks = per-chunk 32-bin sums;
           sorted batch => a 128-node chunk spans < 32 graphs w.h.p.).
  - Host: adds e_in+b2 (exact) and its input-only bincount to totals;
    scatter-adds the 32-wide per-chunk partials into the 16384 graph bins.
"""

import sys

import numpy as np

try:
    import concourse.bass as bass  # noqa: F401
except ImportError:
    sys.path.insert(0, "/opt/trn_rl_repo")

import ml_dtypes
import concourse.bass as bass
import concourse.bacc as bacc
import concourse.tile as tile
from concourse import mybir
from concourse.bass_utils import run_bass_kernel_spmd

BF16 = mybir.dt.bfloat16
F32 = mybir.dt.float32

N_NODES = 500_000
NODE_DIM = 128
HIDDEN = 64
N_GRAPHS = 16_384
N_CORES = 8
NPC = N_NODES // N_CORES          # 62500 nodes per core
CHUNK = 128                       # nodes per chunk (PE contraction width)
MACRO = 2048                      # nodes per macro-tile
CPM = MACRO // CHUNK              # 16 chunks per macro
N_MACRO = (NPC + MACRO - 1) // MACRO   # 31
NPAD = N_MACRO * MACRO            # 63488
N_CHUNK = NPAD // CHUNK           # 496
SPAN = 32                         # max graphs spanned by one sorted 128-chunk


def build_program(level=6, repeat=1):
    """Build the single-core Bass/Tile program (same program on all 8 cores).

    level: cumulative stage count for timeline-sim bisection.
      0: x DMA only; 1: +mm1; 2: +silu; 3: +mm2; 5: +aall; 6: +segmm
    repeat: execute the macro loop N times (for differential device timing).
    """
    nc = bacc.Bacc("TRN2", target_bir_lowering=False, debug=False,
                   num_devices=N_CORES)

    HW = CPM * HIDDEN  # 1024 free elems of h per macro

    # DRAM I/O
    xT = nc.dram_tensor("xT", [NODE_DIM, NPAD], BF16, kind="ExternalInput").ap()
    w1t = nc.dram_tensor("w1t", [NODE_DIM, HIDDEN], BF16, kind="ExternalInput").ap()
    w2rep = nc.dram_tensor("w2rep", [CHUNK, HW], BF16, kind="ExternalInput").ap()
    b1row = nc.dram_tensor("b1row", [1, HW], BF16, kind="ExternalInput").ap()
    ones1 = nc.dram_tensor("ones1", [1, CHUNK], BF16, kind="ExternalInput").ap()
    FP8 = mybir.dt.float8e4
    aoh = nc.dram_tensor("aoh", [CHUNK, N_CHUNK * SPAN], FP8, kind="ExternalInput").ap()
    out_e = nc.dram_tensor("out_e", [CHUNK, N_CHUNK], F32, kind="ExternalOutput").ap()
    out_p = nc.dram_tensor("out_p", [CHUNK, N_CHUNK], F32, kind="ExternalOutput").ap()

    with tile.TileContext(nc) as tc:
        with (
            tc.tile_pool(name="res", bufs=1) as res,
            tc.tile_pool(name="xin", bufs=4) as xin,
            tc.tile_pool(name="silu", bufs=3) as spool,
            tc.tile_pool(name="prod", bufs=3) as ppool,
            tc.tile_pool(name="aall", bufs=3) as apool,
            tc.tile_pool(name="hps", bufs=3, space="PSUM") as hps,
            tc.tile_pool(name="pps", bufs=1, space="PSUM") as pps,
        ):
            # Resident tiles
            t_w1t = res.tile([NODE_DIM, HIDDEN], BF16)
            t_w2rep = res.tile([CHUNK, HW], BF16)
            t_b1row = res.tile([1, HW], BF16)
            t_ones1 = res.tile([1, CHUNK], BF16)
            t_aoh = res.tile([CHUNK, N_CHUNK * SPAN], FP8)
            t_oute = res.tile([CHUNK, N_CHUNK], F32)
            t_psb = res.tile([CHUNK, N_CHUNK], F32)
            psum_p = pps.tile([CHUNK, N_CHUNK], F32)

            nc.sync.dma_start(t_w1t[:], w1t)
            nc.sync.dma_start(t_w2rep[:], w2rep)
            nc.sync.dma_start(t_b1row[:], b1row)
            nc.sync.dma_start(t_ones1[:], ones1)
            for q4 in range(4):
                qs = N_CHUNK * SPAN // 4
                nc.sync.dma_start(t_aoh[:, q4 * qs:(q4 + 1) * qs],
                                  aoh[:, q4 * qs:(q4 + 1) * qs])

            for m in [mm for _ in range(repeat) for mm in range(N_MACRO)]:
                t_x = xin.tile([NODE_DIM, MACRO], BF16)
                nc.sync.dma_start(t_x[:], xT[:, m * MACRO:(m + 1) * MACRO])

                if level < 1:
                    continue
                # h = x @ W1T + b1 into PSUM, nodes on partitions.
                ps_h = hps.tile([CHUNK, HW], F32)
                # rank-1 bias fill: out[p, (c,h)] = ones[p] * b1row[(c,h)]
                # (split at the PSUM bank boundary, N<=512 f32 per matmul)
                for u in range(HW // 512):
                    nc.tensor.matmul(ps_h[:, u * 512:(u + 1) * 512],
                                     t_ones1[:], t_b1row[:, u * 512:(u + 1) * 512],
                                     start=True, stop=False,
                                     skip_group_check=True)
                for c in range(CPM):
                    nc.tensor.matmul(
                        ps_h[:, c * HIDDEN:(c + 1) * HIDDEN],
                        t_x[:, c * CHUNK:(c + 1) * CHUNK],
                        t_w1t[:],
                        start=False, stop=(c == CPM - 1),
                        skip_group_check=True)

                if level < 2:
                    continue
                t_silu = spool.tile([CHUNK, HW], BF16)
                nc.scalar.activation(t_silu[:], ps_h[:],
                                     mybir.ActivationFunctionType.Silu)

                if level < 3:
                    continue
                # e_mlp per node into the resident output tile (e_in + b2 is
                # added on the host; the device segments the MLP part only).
                t_prod = ppool.tile([CHUNK, HW], BF16)
                oute_sl = t_oute[:, m * CPM:(m + 1) * CPM]
                nc.vector.tensor_mul(t_prod[:], t_silu[:], t_w2rep[:])
                nc.vector.tensor_reduce(
                    oute_sl,
                    t_prod[:].rearrange("p (c h) -> p c h", h=HIDDEN),
                    axis=mybir.AxisListType.X, op=mybir.AluOpType.add)

                if level < 5:
                    continue
                # bf16 copy of e for the all-bf16 segment matmul
                t_e16 = apool.tile([CHUNK, CPM], BF16)
                nc.vector.tensor_copy(t_e16[:], oute_sl)

                if level < 6:
                    continue
                # segment partials, 4 chunks per matmul:
                # out[(c,j), c'] = sum_p A[p, (c,j)] e[p, c']
                for q in range(CPM // 4):
                    k0 = m * CPM + 4 * q
                    nc.tensor.matmul(
                        psum_p[:, k0:k0 + 4],
                        t_aoh[:, k0 * SPAN:(k0 + 4) * SPAN],
                        t_e16[:, 4 * q:4 * (q + 1)],
                        start=True, stop=True, skip_group_check=True)

            nc.vector.tensor_copy(t_psb[:], psum_p[:])
            nc.sync.dma_start(out_e, t_oute[:])
            nc.sync.dma_start(out_p, t_psb[:])

    nc.compile()
    return nc


_NC_CACHE = None


def _get_program():
    global _NC_CACHE
    if _NC_CACHE is None:
        _NC_CACHE = build_program()
    return _NC_CACHE


def _prep_core_inputs(xs, bs, W1, b1, W2):
    """Host-side marshalling for one core's shard (xs [NPC,128] f32,
    bs [NPC] int). Returns (in_map, g_base, span_ok)."""
    bf16 = ml_dtypes.bfloat16

    xT = np.zeros((NODE_DIM, NPAD), dtype=bf16)
    xT[:, :NPC] = np.ascontiguousarray(xs.astype(bf16).T)

    rel = np.full(NPAD, -1.0, dtype=np.float32)
    g_base = np.full(N_CHUNK, -1, dtype=np.int64)
    bs64 = bs.astype(np.int64)
    n_valid_chunks = (NPC + CHUNK - 1) // CHUNK
    span_ok = True
    for k in range(n_valid_chunks):
        s = k * CHUNK
        e = min(s + CHUNK, NPC)
        base = bs64[s]
        g_base[k] = base
        r = bs64[s:e] - base
        if r.max() >= SPAN:
            span_ok = False
        rel[s:e] = r.astype(np.float32)
    rel_t = rel.reshape(N_CHUNK, CHUNK).T                      # [128, 496]
    aoh = (rel_t[:, :, None] == np.arange(SPAN, dtype=np.float32)
           ).astype(ml_dtypes.float8_e4m3fn).reshape(CHUNK, N_CHUNK * SPAN)

    w1t = np.ascontiguousarray(W1.T).astype(bf16)               # [128, 64]
    w2row = np.tile(np.asarray(W2).reshape(HIDDEN), CPM)        # [1024]
    w2rep = np.broadcast_to(w2row, (CHUNK, CPM * HIDDEN)).astype(bf16).copy()
    b1row = np.tile(np.asarray(b1), CPM).reshape(1, -1).astype(bf16)
    ones1 = np.ones((1, CHUNK), dtype=bf16)
    in_map = dict(xT=xT, aoh=aoh, w1t=w1t, w2rep=w2rep,
                  b1row=b1row, ones1=ones1)
    return in_map, g_base, span_ok


def kernel(node_scalar, atomic_energies_in, batch, W1, b1, W2, b2):
    node_scalar = np.asarray(node_scalar, dtype=np.float32)
    atomic_energies_in = np.asarray(atomic_energies_in, dtype=np.float32)
    batch = np.asarray(batch)
    W1 = np.asarray(W1, dtype=np.float32)
    b1 = np.asarray(b1, dtype=np.float32)
    W2 = np.asarray(W2, dtype=np.float32)
    b2 = np.asarray(b2, dtype=np.float32)

    nc = _get_program()

    in_maps = []
    g_bases = []
    span_ok_all = True
    for i in range(N_CORES):
        sl = slice(i * NPC, (i + 1) * NPC)
        in_map, g_base, span_ok = _prep_core_inputs(
            node_scalar[sl], batch[sl], W1, b1, W2)
        in_maps.append(in_map)
        g_bases.append(g_base)
        span_ok_all = span_ok_all and span_ok

    res = run_bass_kernel_spmd(nc, in_maps, list(range(N_CORES)))
    results = res.results

    # device computed the MLP term only; e_in + b2 is added here (exact),
    # and its (input-only) segment contribution via bincount.
    ein_b2 = atomic_energies_in + np.float32(b2[0])
    batch64 = batch.astype(np.int64)

    atomic = np.empty(N_NODES, dtype=np.float32)
    total = np.zeros(N_GRAPHS, dtype=np.float64)

    jj = np.arange(SPAN)
    kk = np.arange(N_CHUNK)
    for i in range(N_CORES):
        oe = np.asarray(results[i]["out_e"])        # [128, 496]
        atomic[i * NPC:(i + 1) * NPC] = (
            np.ascontiguousarray(oe.T).reshape(-1)[:NPC])

        if span_ok_all:
            op = np.asarray(results[i]["out_p"])    # [128, 496]
            # partial for chunk k, offset j sits at op[(k % 4)*32 + j, k]
            vals = op[(kk % 4)[:, None] * SPAN + jj[None, :], kk[:, None]]
            gb = g_bases[i]
            valid = gb >= 0
            idx = np.minimum(gb[valid, None] + jj[None, :], N_GRAPHS - 1)
            np.add.at(total, idx.ravel(),
                      vals[valid].astype(np.float64).ravel())

    atomic += ein_b2

    if span_ok_all:
        total += np.bincount(batch64, weights=ein_b2, minlength=N_GRAPHS)
    else:
        # Pathological batch distribution (a 128-node chunk spanning >= 32
        # graphs): host segment-sum of the device-computed atomic energies.
        total = np.zeros(N_GRAPHS, dtype=np.float64)
        np.add.at(total, batch64, atomic.astype(np.float64))

    return atomic, total.astype(np.float32)


if __name__ == "__main__":
    # smoke test vs a local numpy reference
    rng = np.random.default_rng(0)
    x = rng.standard_normal((N_NODES, NODE_DIM), dtype=np.float32)
    ein = rng.standard_normal(N_NODES, dtype=np.float32)
    b = np.sort(rng.integers(0, N_GRAPHS, N_NODES))
    W1_ = rng.uniform(-0.09, 0.09, (HIDDEN, NODE_DIM)).astype(np.float32)
    b1_ = rng.uniform(-0.09, 0.09, HIDDEN).astype(np.float32)
    W2_ = rng.uniform(-0.125, 0.125, (1, HIDDEN)).astype(np.float32)
    b2_ = np.zeros(1, dtype=np.float32)
    a, t = kernel(x, ein, b, W1_, b1_, W2_, b2_)
    h = x @ W1_.T + b1_
    h = h * (1.0 / (1.0 + np.exp(-h)))
    atom = h @ W2_.reshape(-1) + b2_[0] + ein
    tot = np.zeros(N_GRAPHS)
    np.add.at(tot, b, atom)
    print("atomic rel err:", np.abs(a - atom).max() / np.abs(atom).max())
    print("total rel err:", np.abs(t - tot).max() / np.abs(tot).max())
